# revision 27
# baseline (speedup 1.0000x reference)
"""BitConvSwiGLU on 8 Trainium2 cores.

Strategy: pure token-data-parallelism. The 8192 tokens (B*S) are split into
8 slabs of 1024 tokens; each core computes its slab end-to-end (both
matmuls over the full d_hidden) so no collectives are needed. The depthwise
conv needs one halo token on each side, taken from a halo-padded x slab
(zero columns at batch boundaries reproduce the conv's zero padding, since
bit_linear(0) == 0).

v4 design (from the v3 trace: PE busy 251us/314us, GpSimd-bound conv adds,
21us DMA prologue, 2 tscale holes, LDW/HAM inflation on mm1):
- Host pre-scales xs[d,t] = fp16(alpha[t] * xq[d,t]): mm1's PSUM is h
  directly, so the per-chunk dequant multiply, the alpha-row broadcast
  matmuls and their DMAs all disappear (adds ~2^-11 rel err, budget 2e-2).
- mm1 chunks cover BOTH 512-token halves: one w1c stationary load feeds 4
  moving windows (LDW:MM 1:4, fully hidden), and w1 is DMAed once (8.4MB
  -> 8.4MB total, was 16.8).
- GpSimd retired. Conv = 2 fused scalar_tensor_tensor ops on DVE
  ((deq0*r0)+deq1, then (deq2*r2)+u), silu on ACT with scale=cw1/bias=b,
  PSUM evac copies + running absmax on DVE. Per chunk: DVE ~1.3us,
  ACT ~0.9us, vs 3.42us of PE -- consumers never stall the PE.
- Single 8-bank PSUM pool shared by mm1 windows and mm2 accumulators; 2
  mm1 chunks in flight so the PE stream stays dense (HAM stays warm).
- mm2 is n-outer: passes (n, c) share one w2c tile across both halves
  (w2 DMA once, 8.4MB), 8 PSUM banks = 2 halves x 4 token tiles.
- absmax = max(max_c h, 0.2785): silu(z) >= -0.27847 globally, so the
  clamp is exact whenever any channel's h >= 0.2785 - no Abs tracking.
- tscale broadcast via four K=1 matmuls from the transposed scale rows
  (no SBUF->SBUF flatten DMA on the critical path).
- Round-to-int via the +-1.5*2^23 magic-number trick (DVE f32 internal).
"""
import math
from contextlib import ExitStack

import numpy as np
import ml_dtypes


# ---------------------------------------------------------------------------
# Workaround: this walrus build rejects >1 sync wait on CTRL-class
# instructions (Drain/Nop). TileContext's epilogue drain aggregates one wait
# per active proc onto a single Drain. Split the excess onto follow-up nops.
def _install_tile_patch():
    import concourse.mybir as mybir
    from concourse.tile import TileContext
    from concourse.vector_clock import ScopedClock

    if getattr(TileContext, "_drain_patch_installed", False):
        return

    MAX_WAITS = 1

    def _split_waits(nc, inst):
        si = inst.ins.sync_info
        if si is None or len(si.on_wait) <= MAX_WAITS:
            return
        waits = list(si.on_wait)
        si.on_wait = waits[:MAX_WAITS]
        inst.ins.sync_info = si
        for i in range(MAX_WAITS, len(waits), MAX_WAITS):
            nop = nc.sync.nop()
            nop.ins.sync_info = mybir.SyncInfo(
                on_wait=waits[i : i + MAX_WAITS], on_update=[]
            )

    def _patched_drain_and_barrier(self, tick_clock, wait_clock):
        nc = self.nc
        drain_inst = nc.sync.drain()
        wait_clock.add_sem_waits(
            drain_inst.ins, ScopedClock({None: tick_clock.global_clock})
        )
        _split_waits(nc, drain_inst)

        nc.all_engine_barrier()
        assert self.sems is not None
        popped = nc._tile_sem_poison_stack.pop()
        assert popped is self._sem_poison
        nc.clear_and_free_semaphores(list(self.sems.allocated().values()))
        nc.all_engine_barrier()

    TileContext._drain_and_barrier = _patched_drain_and_barrier
    TileContext._drain_patch_installed = True

    # Generic safety net: rewrite the BIR JSON before compile, splitting any
    # instruction with >1 sync wait into same-engine NoOps placed before it
    # (a same-engine nop stalls the engine identically, so semantics hold).
    import json as _json
    import concourse.bass_utils as _bu
    import concourse.bass2jax as _b2j

    _orig_compile = _bu.compile_bir_kernel

    def _split_bir_waits(bir_json: bytes) -> bytes:
        d = _json.loads(bir_json)
        n_split = [0]

        def fix_block(b):
            insts = b.get("instructions", [])
            out = []
            for inst in insts:
                si = inst.get("sync_info")
                waits = si.get("on_wait") if si else None
                if waits and len(waits) > 1:
                    keep, extra = waits[:1], waits[1:]
                    for j in range(0, len(extra)):
                        out.append({
                            "name": f"{inst['name']}_w{j}",
                            "opcode": "NoOp",
                            "engine": inst.get("engine", "SP"),
                            "ins": [],
                            "outs": [],
                            "sync_info": {
                                "on_wait": [extra[j]],
                                "on_update": [],
                            },
                        })
                        n_split[0] += 1
                    si["on_wait"] = keep
                out.append(inst)
            b["instructions"] = out
            for sub in b.get("blocks", []):
                fix_block(sub)

        for f in d.get("functions", []):
            for b in f.get("blocks", []):
                fix_block(b)
        if n_split[0]:
            return _json.dumps(d).encode()
        return bir_json

    def _patched_compile(bir_json, tmpdir, neff_name="file.neff"):
        return _orig_compile(_split_bir_waits(bir_json), tmpdir, neff_name)

    _bu.compile_bir_kernel = _patched_compile
    _b2j.compile_bir_kernel = _patched_compile


# ---------------------------------------------------------------------------
# Problem dims (hardcoded per contract)
B, S, D, H = 4, 2048, 1024, 4096
N_CORES = 8
EPS = 1e-5
P = 128
MAGIC = 12582912.0  # 1.5 * 2**23: f32 addend that forces round-to-nearest-int
SILU_MIN = 0.2785   # > |global min of silu| = 0.27847; absmax clamp floor


def build_nc(t_own, beta_c):
    """Build the SPMD single-core program for a slab of t_own tokens."""
    import concourse.bass as bass
    import concourse.mybir as mybir
    from concourse.tile import TileContext
    from concourse.masks import make_identity

    f32 = mybir.dt.float32
    fp16 = mybir.dt.float16
    AF = mybir.ActivationFunctionType
    ALU = mybir.AluOpType
    AX = mybir.AxisListType

    assert t_own % 256 == 0
    half = t_own // 2        # 512 own tokens per half
    hext = half + 2          # 514: + conv halo
    W = hext // 2            # 257: mm1/PSUM window
    text = t_own + 2         # 1026 extended tokens
    dc = D // P              # 8
    cc = H // P              # 32
    mt = half // P           # 4 output token tiles per half

    nc = bass.Bass()
    xst_d = nc.declare_dram_parameter("xst", [P, D // P, text], fp16,
                                      isOutput=False)
    w1s = nc.declare_dram_parameter("w1s", [cc, P, D], fp16, isOutput=False)
    w2t = nc.declare_dram_parameter("w2t", [H, D], fp16, isOutput=False)
    cwal = nc.declare_dram_parameter("cwal", [P, cc * 4], f32, isOutput=False)
    oh4_d = nc.declare_dram_parameter("oh4", [t_own // 256, t_own // 2], f32,
                                      isOutput=False)
    y_out = nc.declare_dram_parameter("y", [t_own, D], f32, isOutput=True)

    ctx = ExitStack()
    with TileContext(nc) as tc, ctx:
        pool = lambda name, bufs, space="SBUF": ctx.enter_context(
            tc.tile_pool(name=name, bufs=bufs, space=space)
        )
        const = pool("const", 1)
        xs_pool = pool("xs", 1)
        w1p = pool("w1p", 6)
        w2p = pool("w2p", 32)
        deqp = pool("deq", 6)
        tmpp = pool("tmp", 8)
        hp = [pool("h0", cc), pool("h1", cc)]
        stats = pool("stats", 2)
        ysb_p = pool("ysb", 6)
        ps = pool("ps", 8, "PSUM")

        # x + first w1 chunk DMAs first: they gate the PE start
        xsq = []
        for q in range(8):
            t = xs_pool.tile([P, 1, text], fp16, tag=f"xsq{q}",
                             name=f"xsq{q}")
            xsq.append(t)
        w1tiles = {}

        def w1_fetch(c):
            w1c = w1p.tile([P, dc, P], fp16, tag="w1c", name=f"w1c{c}")
            nc.sync.dma_start(
                out=w1c[:], in_=w1s[c].rearrange("p (k m) -> p k m", k=dc)
            )
            w1tiles[c] = w1c

        nc.sync.dma_start(out=xsq[0][:], in_=xst_d[:, 0:1, :])
        w1_fetch(0)
        w1_fetch(1)
        for q in range(1, 8):
            nc.sync.dma_start(out=xsq[q][:],
                              in_=xst_d[:, q : q + 1, :])

        WA = W + 1   # 258: even window offsets keep DVE/PE APs 4B-aligned

        def xs_mv(hf, d, wi):
            base = hf * half + wi * (WA - 2)
            return xsq[d][:, 0, base : base + WA]

        ident_h = const.tile([P, P], fp16, tag="idh")
        make_identity(nc, ident_h)
        ident_f = const.tile([P, P], f32, tag="idf")
        make_identity(nc, ident_f)
        # oh4[:, m*128:(m+1)*128] is one-hot row m: K=4 matmuls against the
        # transposed scale rows broadcast each row across all 128 partitions.
        oh4 = const.tile([mt, mt * P], f32, tag="oh4")
        nc.sync.dma_start(out=oh4[:], in_=oh4_d[:, :])

        cwres = const.tile([P, cc * 4], f32, tag="cw")
        nc.sync.dma_start(out=cwres[:], in_=cwal[:, :])

        # ---------------- stage 0: pre-scaled x load -----------------------
        # Partition-major DRAM layout -> contiguous 16KB-per-partition
        # descriptors; two dma_starts for queue parallelism.


        # ---------------- per-chunk mm1 + conv + silu ----------------------
        h_tiles = [[None] * cc, [None] * cc]
        hq_tiles = [[None] * cc, [None] * cc]
        maccs = []
        for hf in range(2):
            macc = const.tile([P, half], fp16, tag=f"macc{hf}")
            nc.any.memset(macc[:], 0.0)
            maccs.append(macc)

        def mm1_chunk(c, hfs, keep_w1=False):
            r0 = cwres[:, 4 * c + 0 : 4 * c + 1]
            r2 = cwres[:, 4 * c + 1 : 4 * c + 2]
            cw1 = cwres[:, 4 * c + 2 : 4 * c + 3]
            cwb = cwres[:, 4 * c + 3 : 4 * c + 4]
            if c not in w1tiles:
                w1_fetch(c)
            w1c = w1tiles[c] if keep_w1 else w1tiles.pop(c)
            pms = {
                (hf, wi): ps.tile([P, W + 1], f32, tag="ps",
                                  name=f"pm{c}_{hf}_{wi}")
                for hf in hfs for wi in range(2)
            }
            for d in range(dc):
                for hf in hfs:
                    for wi in range(2):
                        nc.tensor.matmul(
                            pms[(hf, wi)][:], w1c[:, d, :],
                            xs_mv(hf, d, wi),
                            start=(d == 0), stop=(d == dc - 1),
                        )
            for hf in hfs:
                deq = deqp.tile([P, hext], fp16, tag="deq")
                nc.scalar.activation(deq[:, 0 : W + 1], pms[(hf, 0)][:],
                                     AF.Copy)
                nc.scalar.activation(deq[:, W + 1 : hext],
                                     pms[(hf, 1)][:, 2 : W + 1], AF.Copy)
                u = tmpp.tile([P, half], fp16, tag="u")
                nc.vector.scalar_tensor_tensor(
                    u[:], deq[:, 0:half], r0, deq[:, 1 : 1 + half],
                    op0=ALU.mult, op1=ALU.add,
                )
                s2 = tmpp.tile([P, half], fp16, tag="s2")
                nc.vector.scalar_tensor_tensor(
                    s2[:], deq[:, 2 : 2 + half], r2, u[:],
                    op0=ALU.mult, op1=ALU.add,
                )
                h = hp[hf].tile([P, half], fp16, tag="h", name=f"h{hf}_{c}")
                nc.scalar.activation(h[:], s2[:], AF.Silu, scale=cw1,
                                     bias=cwb)
                nc.vector.tensor_tensor(maccs[hf][:], maccs[hf][:], h[:],
                                        op=ALU.max)
                h_tiles[hf][c] = h

        # -------- per-half token scales (generator: 2 emission phases) ------
        def tscale(hf):
            mh = stats.tile([P, mt], f32, tag="mh", name=f"mh{hf}")
            for m in range(mt):
                pt = ps.tile([P, P], fp16, tag="ps", name=f"pt{hf}_{m}")
                nc.tensor.transpose(
                    pt[:], maccs[hf][:, m * P : (m + 1) * P], ident_h[:]
                )
                nc.vector.tensor_reduce(mh[:, m : m + 1], pt[:],
                                        axis=AX.X, op=ALU.max)
            yield None, None
            nc.vector.tensor_scalar_max(mh[:], mh[:], SILU_MIN)
            beta_cols = stats.tile([P, mt], f32, tag="bcols",
                                   name=f"bcols{hf}")
            nc.vector.tensor_scalar_mul(beta_cols[:], mh[:], beta_c)
            shcols = stats.tile([P, mt], f32, tag="shcols",
                                name=f"shcols{hf}")
            nc.vector.reciprocal(shcols[:], mh[:])
            spt = ps.tile([mt, P], f32, tag="ps", name=f"spt{hf}")
            nc.tensor.transpose(spt[:], shcols[:], ident_f[:])
            sh4 = stats.tile([mt, P], f32, tag="sh4", name=f"sh4{hf}")
            nc.vector.tensor_copy(sh4[:], spt[:])
            pb = ps.tile([P, half], f32, tag="ps", name=f"pb{hf}")
            for m in range(mt):
                nc.tensor.matmul(
                    pb[:, m * P : (m + 1) * P],
                    oh4[:, m * P : (m + 1) * P], sh4[:],
                    start=True, stop=True,
                )
            shbc = stats.tile([P, half], fp16, tag="shbc", name=f"shbc{hf}")
            nc.vector.tensor_copy(shbc[:], pb[:])
            yield beta_cols, shbc

        def quant_chunk(hf, c, shbc):
            h = h_tiles[hf][c]
            prod = tmpp.tile([P, half], fp16, tag="qp")
            nc.vector.tensor_tensor(prod[:], h[:], shbc[:], op=ALU.mult)
            hq = hp[hf].tile([P, half], fp16, tag="h", name=f"hq{hf}_{c}")
            nc.vector.tensor_scalar(hq[:], prod[:], MAGIC, -MAGIC,
                                    op0=ALU.add, op1=ALU.add)
            hq_tiles[hf][c] = hq

        def w2_load(n, c):
            w2c = w2p.tile([P, 512], fp16, tag="w2c")
            nc.sync.dma_start(
                out=w2c[:],
                in_=w2t[c * P : (c + 1) * P, n * 512 : (n + 1) * 512],
            )
            return w2c

        def ysb_out(psy, betas, n, hf, m, on_act):
            ysb = ysb_p.tile([P, 512], f32, tag="ysb")
            if on_act:
                nc.scalar.activation(ysb[:], psy[:], AF.Copy,
                                     scale=betas[hf][:, m : m + 1])
            else:
                nc.vector.tensor_scalar_mul(ysb[:], psy[:],
                                            betas[hf][:, m : m + 1])
            nc.sync.dma_start(
                out=y_out[hf * half + m * P : hf * half + (m + 1) * P,
                          n * 512 : (n + 1) * 512],
                in_=ysb[:],
            )

        # mm2 pass 0, one half: c-outer; w2c tiles preloaded and shared
        def mm2_pass0_half(w2cs, betas, hf):
            psy = [ps.tile([P, 512], f32, tag="ps", name=f"psy0_{hf}_{m}")
                   for m in range(mt)]
            for c in range(cc):
                hq = hq_tiles[hf][c]
                for m in range(mt):
                    nc.tensor.matmul(
                        psy[m][:], hq[:, m * P : (m + 1) * P],
                        w2cs[c][:], start=(c == 0), stop=(c == cc - 1),
                    )
            for m in range(mt):
                ysb_out(psy[m][:], betas, 0, hf, m, on_act=(m % 2 == 0))

        # ---------------- schedule ------------------------------------------
        # Back-split: half-0 finishes TAIL chunks early; tscale(0) PE bits
        # interleave into half-1's tail so its chain hides under matmuls.
        # mm2 pass 0 runs the halves as separate c-outer blocks: tscale(1)
        # and the quant streams hide under the blocks' matmuls.
        TAIL = 4
        for c in range(cc - TAIL):
            mm1_chunk(c, [0, 1])
        for c in range(cc - TAIL, cc):
            mm1_chunk(c, [0], keep_w1=True)
        t0_gen = tscale(0)
        w2cs0 = [w2_load(0, c) for c in range(cc)]
        for i, c in enumerate(range(cc - TAIL, cc)):
            mm1_chunk(c, [1])
            if i == 0:
                next(t0_gen)
            elif i == 1:
                beta0, shbc0 = next(t0_gen)
            elif i == 2:
                for q in range(3):
                    quant_chunk(0, q, shbc0)
            else:
                for q in range(3, 6):
                    quant_chunk(0, q, shbc0)
        betas = {0: beta0, 1: None}
        # pass 0, half 0
        psy0 = [ps.tile([P, 512], f32, tag="ps", name=f"psy0_0_{m}")
                for m in range(mt)]
        t1_gen = tscale(1)
        shbc1 = None
        for c in range(cc):
            for m in range(mt):
                nc.tensor.matmul(
                    psy0[m][:], hq_tiles[0][c][:, m * P : (m + 1) * P],
                    w2cs0[c][:], start=(c == 0), stop=(c == cc - 1),
                )
            if c == 0:
                next(t1_gen)
            elif c == 1:
                beta1, shbc1 = next(t1_gen)
                betas[1] = beta1
            elif c - 2 + 6 < cc:
                quant_chunk(0, c - 2 + 6, shbc0)
            elif c >= cc - 4:
                quant_chunk(1, c - (cc - 4), shbc1)
        for m in range(mt):
            ysb_out(psy0[m][:], betas, 0, 0, m, on_act=True)
        # pass 0, half 1, quant(1) interleaved at lag 4
        psy1 = [ps.tile([P, 512], f32, tag="ps", name=f"psy0_1_{m}")
                for m in range(mt)]
        for c in range(cc):
            for m in range(mt):
                nc.tensor.matmul(
                    psy1[m][:], hq_tiles[1][c][:, m * P : (m + 1) * P],
                    w2cs0[c][:], start=(c == 0), stop=(c == cc - 1),
                )
            if c + 4 < cc:
                quant_chunk(1, c + 4, shbc1)
        for m in range(mt):
            ysb_out(psy1[m][:], betas, 0, 1, m, on_act=True)
        # pass 1: m-paired groups (4 PSUM banks each, short output tail)
        w2cs = [w2_load(1, c) for c in range(cc)]
        for mp in range(2):
            psy = [[ps.tile([P, 512], f32, tag="ps",
                            name=f"psy1_{hf}_{2 * mp + mi}")
                    for mi in range(2)] for hf in range(2)]
            for c in range(cc):
                for hf in range(2):
                    for mi in range(2):
                        m = 2 * mp + mi
                        nc.tensor.matmul(
                            psy[hf][mi][:],
                            hq_tiles[hf][c][:, m * P : (m + 1) * P],
                            w2cs[c][:], start=(c == 0), stop=(c == cc - 1),
                        )
            for hf in range(2):
                for mi in range(2):
                    ysb_out(psy[hf][mi][:], betas, 1, hf, 2 * mp + mi,
                            on_act=(hf == 0))
    return nc


def _host_prep(x, w1, conv_w, conv_b, w2, t_own):
    """Quantize weights and build per-core halo-padded pre-scaled x slabs."""
    fp16 = np.float16
    cc, dc = H // P, D // P
    s1inv = np.maximum(np.mean(np.abs(w1)), np.float32(EPS)).astype(np.float32)
    w1q = np.clip(np.rint(w1 * (np.float32(1.0) / s1inv)), -1, 1).astype(
        np.float32
    )
    s2inv = np.maximum(np.mean(np.abs(w2)), np.float32(EPS)).astype(np.float32)
    w2q = np.clip(np.rint(w2 * (np.float32(1.0) / s2inv)), -1, 1).astype(
        np.float32
    )

    # w1s[c, p, k*128+m] = w1q[c*128+m, k*128+p] -> per-chunk contiguous lhsT
    w1s = np.ascontiguousarray(
        w1q.reshape(cc, P, dc, P).transpose(0, 3, 2, 1).reshape(cc, P, D)
    ).astype(fp16)
    w2t = np.ascontiguousarray(w2q.T).astype(fp16)          # [H, D]
    cw0 = conv_w[:, 0, 0].astype(np.float32)
    cw1 = conv_w[:, 0, 1].astype(np.float32)
    cw2 = conv_w[:, 0, 2].astype(np.float32)
    # folded conv: conv = cw1*(deq1 + r0*deq0 + r2*deq2); silu scale = cw1
    r0 = cw0 / cw1
    r2 = cw2 / cw1
    # overflow guard (DVE computes stt in f32 internally; this only bounds
    # the f32 products; error negligible since the corresponding cw0/cw2
    # contribution is then ~unchanged)
    lim = np.float32(2.0e4 * 3.0)
    r0 = np.clip(r0, -lim, lim)
    r2 = np.clip(r2, -lim, lim)
    cw = np.stack([r0, r2, cw1, conv_b.astype(np.float32)], axis=1)
    cwal = np.ascontiguousarray(
        cw.reshape(cc, P, 4).transpose(1, 0, 2).reshape(P, cc * 4)
    ).astype(np.float32)

    n_cores = x.shape[0] * x.shape[1] // t_own
    xf = x.reshape(-1, x.shape[-1]).astype(np.float32)
    am = np.abs(xf).max(axis=1, keepdims=True).astype(np.float32)
    amc = np.maximum(am, np.float32(EPS))
    sxv = (np.float32(1.0) / amc).astype(np.float32) * np.float32(127.0)
    xq = np.rint((xf * sxv).astype(np.float32)).astype(np.float32)
    alpha_row = (amc[:, 0] * np.float32(s1inv / 127.0)).astype(np.float32)
    xsc = (xq * alpha_row[:, None]).astype(fp16)   # pre-scaled activations
    slabs = []
    for c in range(n_cores):
        lo = c * t_own
        xe = np.zeros((t_own + 2, xf.shape[1]), fp16)
        xe[1 : 1 + t_own] = xsc[lo : lo + t_own]
        if lo % S != 0:
            xe[0] = xsc[lo - 1]
        if (lo + t_own) % S != 0 and lo + t_own < xf.shape[0]:
            xe[1 + t_own] = xsc[lo + t_own]
        xt = xe.T.reshape(dc, P, t_own + 2).transpose(1, 0, 2)
        slabs.append(np.ascontiguousarray(xt))
    beta_c = float(s2inv) / 127.0
    return w1s, w2t, cwal, slabs, beta_c


def _run(x, w1, conv_w, conv_b, w2, trace=False, **spmd_kwargs):
    import sys
    if "/opt/trn_rl_repo" not in sys.path:
        sys.path.append("/opt/trn_rl_repo")
    _install_tile_patch()
    from concourse.bass_utils import run_bass_kernel_spmd

    t_own = x.shape[0] * x.shape[1] // N_CORES
    w1s, w2t, cwal, slabs, beta_c = _host_prep(
        x, w1, conv_w, conv_b, w2, t_own
    )
    nc = build_nc(t_own, beta_c)
    mt = t_own // 2 // 128
    oh4 = np.zeros((mt, mt * 128), np.float32)
    for m in range(mt):
        oh4[m, m * 128 : (m + 1) * 128] = 127.0
    in_maps = [
        {"xst": slabs[c], "w1s": w1s, "w2t": w2t, "cwal": cwal, "oh4": oh4}
        for c in range(N_CORES)
    ]
    out = run_bass_kernel_spmd(
        nc, in_maps, list(range(N_CORES)), trace=trace, **spmd_kwargs
    )
    y = np.concatenate([out.results[c]["y"] for c in range(N_CORES)], axis=0)
    y = np.ascontiguousarray(y.reshape(x.shape[0], x.shape[1], -1))
    return y, out


def kernel(x, w1, conv_w, conv_b, w2):
    return _run(x, w1, conv_w, conv_b, w2)[0]


# revision 28
# speedup vs baseline: 1.2038x; 1.2038x over previous
"""BitConvSwiGLU on 8 Trainium2 cores.

Strategy: pure token-data-parallelism. The 8192 tokens (B*S) are split into
8 slabs of 1024 tokens; each core computes its slab end-to-end (both
matmuls over the full d_hidden) so no collectives are needed. The depthwise
conv needs one halo token on each side, taken from a halo-padded x slab
(zero columns at batch boundaries reproduce the conv's zero padding, since
bit_linear(0) == 0).

v4 design (from the v3 trace: PE busy 251us/314us, GpSimd-bound conv adds,
21us DMA prologue, 2 tscale holes, LDW/HAM inflation on mm1):
- Host pre-scales xs[d,t] = fp16(alpha[t] * xq[d,t]): mm1's PSUM is h
  directly, so the per-chunk dequant multiply, the alpha-row broadcast
  matmuls and their DMAs all disappear (adds ~2^-11 rel err, budget 2e-2).
- mm1 chunks cover BOTH 512-token halves: one w1c stationary load feeds 4
  moving windows (LDW:MM 1:4, fully hidden), and w1 is DMAed once (8.4MB
  -> 8.4MB total, was 16.8).
- GpSimd retired. Conv = 2 fused scalar_tensor_tensor ops on DVE
  ((deq0*r0)+deq1, then (deq2*r2)+u), silu on ACT with scale=cw1/bias=b,
  PSUM evac copies + running absmax on DVE. Per chunk: DVE ~1.3us,
  ACT ~0.9us, vs 3.42us of PE -- consumers never stall the PE.
- Single 8-bank PSUM pool shared by mm1 windows and mm2 accumulators; 2
  mm1 chunks in flight so the PE stream stays dense (HAM stays warm).
- mm2 is n-outer: passes (n, c) share one w2c tile across both halves
  (w2 DMA once, 8.4MB), 8 PSUM banks = 2 halves x 4 token tiles.
- absmax = max(max_c h, 0.2785): silu(z) >= -0.27847 globally, so the
  clamp is exact whenever any channel's h >= 0.2785 - no Abs tracking.
- tscale broadcast via four K=1 matmuls from the transposed scale rows
  (no SBUF->SBUF flatten DMA on the critical path).
- Round-to-int via the +-1.5*2^23 magic-number trick (DVE f32 internal).
"""
import math
from contextlib import ExitStack

import numpy as np
import ml_dtypes


# ---------------------------------------------------------------------------
# Workaround: this walrus build rejects >1 sync wait on CTRL-class
# instructions (Drain/Nop). TileContext's epilogue drain aggregates one wait
# per active proc onto a single Drain. Split the excess onto follow-up nops.
def _install_tile_patch():
    import concourse.mybir as mybir
    from concourse.tile import TileContext
    from concourse.vector_clock import ScopedClock

    if getattr(TileContext, "_drain_patch_installed", False):
        return

    MAX_WAITS = 1

    def _split_waits(nc, inst):
        si = inst.ins.sync_info
        if si is None or len(si.on_wait) <= MAX_WAITS:
            return
        waits = list(si.on_wait)
        si.on_wait = waits[:MAX_WAITS]
        inst.ins.sync_info = si
        for i in range(MAX_WAITS, len(waits), MAX_WAITS):
            nop = nc.sync.nop()
            nop.ins.sync_info = mybir.SyncInfo(
                on_wait=waits[i : i + MAX_WAITS], on_update=[]
            )

    def _patched_drain_and_barrier(self, tick_clock, wait_clock):
        nc = self.nc
        drain_inst = nc.sync.drain()
        wait_clock.add_sem_waits(
            drain_inst.ins, ScopedClock({None: tick_clock.global_clock})
        )
        _split_waits(nc, drain_inst)

        nc.all_engine_barrier()
        assert self.sems is not None
        popped = nc._tile_sem_poison_stack.pop()
        assert popped is self._sem_poison
        nc.clear_and_free_semaphores(list(self.sems.allocated().values()))
        nc.all_engine_barrier()

    TileContext._drain_and_barrier = _patched_drain_and_barrier
    TileContext._drain_patch_installed = True

    # Generic safety net: rewrite the BIR JSON before compile, splitting any
    # instruction with >1 sync wait into same-engine NoOps placed before it
    # (a same-engine nop stalls the engine identically, so semantics hold).
    import json as _json
    import concourse.bass_utils as _bu
    import concourse.bass2jax as _b2j

    _orig_compile = _bu.compile_bir_kernel

    def _split_bir_waits(bir_json: bytes) -> bytes:
        d = _json.loads(bir_json)
        n_split = [0]

        def fix_block(b):
            insts = b.get("instructions", [])
            out = []
            for inst in insts:
                si = inst.get("sync_info")
                waits = si.get("on_wait") if si else None
                if waits and len(waits) > 1:
                    keep, extra = waits[:1], waits[1:]
                    for j in range(0, len(extra)):
                        out.append({
                            "name": f"{inst['name']}_w{j}",
                            "opcode": "NoOp",
                            "engine": inst.get("engine", "SP"),
                            "ins": [],
                            "outs": [],
                            "sync_info": {
                                "on_wait": [extra[j]],
                                "on_update": [],
                            },
                        })
                        n_split[0] += 1
                    si["on_wait"] = keep
                out.append(inst)
            b["instructions"] = out
            for sub in b.get("blocks", []):
                fix_block(sub)

        for f in d.get("functions", []):
            for b in f.get("blocks", []):
                fix_block(b)
        if n_split[0]:
            return _json.dumps(d).encode()
        return bir_json

    def _patched_compile(bir_json, tmpdir, neff_name="file.neff"):
        return _orig_compile(_split_bir_waits(bir_json), tmpdir, neff_name)

    _bu.compile_bir_kernel = _patched_compile
    _b2j.compile_bir_kernel = _patched_compile


# ---------------------------------------------------------------------------
# Problem dims (hardcoded per contract)
B, S, D, H = 4, 2048, 1024, 4096
N_CORES = 8
EPS = 1e-5
P = 128
MAGIC = 12582912.0  # 1.5 * 2**23: f32 addend that forces round-to-nearest-int
SILU_MIN = 0.2785   # > |global min of silu| = 0.27847; absmax clamp floor


def build_nc(t_own, beta_c):
    """Build the SPMD single-core program for a slab of t_own tokens."""
    import concourse.bass as bass
    import concourse.mybir as mybir
    from concourse.tile import TileContext
    from concourse.masks import make_identity

    f32 = mybir.dt.float32
    fp16 = mybir.dt.float16
    AF = mybir.ActivationFunctionType
    ALU = mybir.AluOpType
    AX = mybir.AxisListType

    assert t_own % 256 == 0
    half = t_own // 2        # 512 own tokens per half
    hext = half + 2          # 514: + conv halo
    W = hext // 2            # 257: mm1/PSUM window
    text = t_own + 2         # 1026 extended tokens
    dc = D // P              # 8
    cc = H // P              # 32
    mt = half // P           # 4 output token tiles per half

    nc = bass.Bass()
    xst_d = nc.declare_dram_parameter("xst", [P, D // P, text], fp16,
                                      isOutput=False)
    w1s = nc.declare_dram_parameter("w1s", [cc, P, D], fp16, isOutput=False)
    w2t = nc.declare_dram_parameter("w2t", [H, D], fp16, isOutput=False)
    cwal = nc.declare_dram_parameter("cwal", [P, cc * 4], f32, isOutput=False)
    oh4_d = nc.declare_dram_parameter("oh4", [t_own // 256, t_own // 2], f32,
                                      isOutput=False)
    y_out = nc.declare_dram_parameter("y", [t_own, D], f32, isOutput=True)

    ctx = ExitStack()
    with TileContext(nc) as tc, ctx:
        pool = lambda name, bufs, space="SBUF": ctx.enter_context(
            tc.tile_pool(name=name, bufs=bufs, space=space)
        )
        const = pool("const", 1)
        xs_pool = pool("xs", 1)
        w1p = pool("w1p", 4)
        w2p = pool("w2p", 32)
        deqp = pool("deq", 6)
        tmpp = pool("tmp", 8)
        hp = [pool("h0", cc), pool("h1", cc)]
        stats = pool("stats", 2)
        ysb_p = pool("ysb", 6)
        ps = pool("ps", 8, "PSUM")

        # x + first w1 chunk DMAs first: they gate the PE start
        xsq = []
        for q in range(8):
            t = xs_pool.tile([P, 1, text], fp16, tag=f"xsq{q}",
                             name=f"xsq{q}")
            xsq.append(t)
        w1tiles = {}

        def w1_fetch(c):
            w1c = w1p.tile([P, dc, P], fp16, tag="w1c", name=f"w1c{c}")
            nc.sync.dma_start(
                out=w1c[:], in_=w1s[c].rearrange("p (k m) -> p k m", k=dc)
            )
            w1tiles[c] = w1c

        nc.sync.dma_start(out=xsq[0][:], in_=xst_d[:, 0:1, :])
        w1_fetch(0)
        w1_fetch(1)
        for q in range(1, 8):
            nc.sync.dma_start(out=xsq[q][:],
                              in_=xst_d[:, q : q + 1, :])

        WA = W + 1   # 258: even window offsets keep DVE/PE APs 4B-aligned

        def xs_mv(hf, d, wi):
            base = hf * half + wi * (WA - 2)
            return xsq[d][:, 0, base : base + WA]

        ident_h = const.tile([P, P], fp16, tag="idh")
        make_identity(nc, ident_h)
        ident_f = const.tile([P, P], f32, tag="idf")
        make_identity(nc, ident_f)
        # oh4[:, m*128:(m+1)*128] is one-hot row m: K=4 matmuls against the
        # transposed scale rows broadcast each row across all 128 partitions.
        oh4 = const.tile([mt, mt * P], f32, tag="oh4")
        nc.sync.dma_start(out=oh4[:], in_=oh4_d[:, :])

        cwres = const.tile([P, cc * 4], f32, tag="cw")
        nc.sync.dma_start(out=cwres[:], in_=cwal[:, :])

        # ---------------- stage 0: pre-scaled x load -----------------------
        # Partition-major DRAM layout -> contiguous 16KB-per-partition
        # descriptors; two dma_starts for queue parallelism.


        # ---------------- per-chunk mm1 + conv + silu ----------------------
        h_tiles = [[None] * cc, [None] * cc]
        hq_tiles = [[None] * cc, [None] * cc]
        maccs = []
        for hf in range(2):
            macc = const.tile([P, half], fp16, tag=f"macc{hf}")
            nc.any.memset(macc[:], 0.0)
            maccs.append(macc)

        def mm1_chunk(c, hfs, keep_w1=False):
            r0 = cwres[:, 4 * c + 0 : 4 * c + 1]
            r2 = cwres[:, 4 * c + 1 : 4 * c + 2]
            cw1 = cwres[:, 4 * c + 2 : 4 * c + 3]
            cwb = cwres[:, 4 * c + 3 : 4 * c + 4]
            if c not in w1tiles:
                w1_fetch(c)
            w1c = w1tiles[c] if keep_w1 else w1tiles.pop(c)
            pms = {
                (hf, wi): ps.tile([P, W + 1], f32, tag="ps",
                                  name=f"pm{c}_{hf}_{wi}")
                for hf in hfs for wi in range(2)
            }
            for d in range(dc):
                for hf in hfs:
                    for wi in range(2):
                        nc.tensor.matmul(
                            pms[(hf, wi)][:], w1c[:, d, :],
                            xs_mv(hf, d, wi),
                            start=(d == 0), stop=(d == dc - 1),
                        )
            for hf in hfs:
                deq = deqp.tile([P, hext], fp16, tag="deq")
                nc.scalar.activation(deq[:, 0 : W + 1], pms[(hf, 0)][:],
                                     AF.Copy)
                nc.scalar.activation(deq[:, W + 1 : hext],
                                     pms[(hf, 1)][:, 2 : W + 1], AF.Copy)
                u = tmpp.tile([P, half], fp16, tag="u")
                nc.vector.scalar_tensor_tensor(
                    u[:], deq[:, 0:half], r0, deq[:, 1 : 1 + half],
                    op0=ALU.mult, op1=ALU.add,
                )
                s2 = tmpp.tile([P, half], fp16, tag="s2")
                nc.vector.scalar_tensor_tensor(
                    s2[:], deq[:, 2 : 2 + half], r2, u[:],
                    op0=ALU.mult, op1=ALU.add,
                )
                h = hp[hf].tile([P, half], fp16, tag="h", name=f"h{hf}_{c}")
                nc.scalar.activation(h[:], s2[:], AF.Silu, scale=cw1,
                                     bias=cwb)
                nc.vector.tensor_tensor(maccs[hf][:], maccs[hf][:], h[:],
                                        op=ALU.max)
                h_tiles[hf][c] = h

        # -------- per-half token scales (generator: 2 emission phases) ------
        def tscale(hf):
            mh = stats.tile([P, mt], f32, tag="mh", name=f"mh{hf}")
            for m in range(mt):
                pt = ps.tile([P, P], fp16, tag="ps", name=f"pt{hf}_{m}")
                nc.tensor.transpose(
                    pt[:], maccs[hf][:, m * P : (m + 1) * P], ident_h[:]
                )
                nc.vector.tensor_reduce(mh[:, m : m + 1], pt[:],
                                        axis=AX.X, op=ALU.max)
            yield None, None
            nc.vector.tensor_scalar_max(mh[:], mh[:], SILU_MIN)
            beta_cols = stats.tile([P, mt], f32, tag="bcols",
                                   name=f"bcols{hf}")
            nc.vector.tensor_scalar_mul(beta_cols[:], mh[:], beta_c)
            shcols = stats.tile([P, mt], f32, tag="shcols",
                                name=f"shcols{hf}")
            nc.vector.reciprocal(shcols[:], mh[:])
            spt = ps.tile([mt, P], f32, tag="ps", name=f"spt{hf}")
            nc.tensor.transpose(spt[:], shcols[:], ident_f[:])
            sh4 = stats.tile([mt, P], f32, tag="sh4", name=f"sh4{hf}")
            nc.vector.tensor_copy(sh4[:], spt[:])
            pb = ps.tile([P, half], f32, tag="ps", name=f"pb{hf}")
            for m in range(mt):
                nc.tensor.matmul(
                    pb[:, m * P : (m + 1) * P],
                    oh4[:, m * P : (m + 1) * P], sh4[:],
                    start=True, stop=True,
                )
            shbc = stats.tile([P, half], fp16, tag="shbc", name=f"shbc{hf}")
            nc.vector.tensor_copy(shbc[:], pb[:])
            yield beta_cols, shbc

        def quant_chunk(hf, c, shbc):
            h = h_tiles[hf][c]
            prod = tmpp.tile([P, half], fp16, tag="qp")
            nc.vector.tensor_tensor(prod[:], h[:], shbc[:], op=ALU.mult)
            hq = hp[hf].tile([P, half], fp16, tag="h", name=f"hq{hf}_{c}")
            nc.vector.tensor_scalar(hq[:], prod[:], MAGIC, -MAGIC,
                                    op0=ALU.add, op1=ALU.add)
            hq_tiles[hf][c] = hq

        def w2_load(n, c):
            w2c = w2p.tile([P, 512], fp16, tag="w2c")
            nc.sync.dma_start(
                out=w2c[:],
                in_=w2t[c * P : (c + 1) * P, n * 512 : (n + 1) * 512],
            )
            return w2c

        def ysb_out(psy, betas, n, hf, m, on_act):
            ysb = ysb_p.tile([P, 512], f32, tag="ysb")
            if on_act:
                nc.scalar.activation(ysb[:], psy[:], AF.Copy,
                                     scale=betas[hf][:, m : m + 1])
            else:
                nc.vector.tensor_scalar_mul(ysb[:], psy[:],
                                            betas[hf][:, m : m + 1])
            nc.sync.dma_start(
                out=y_out[hf * half + m * P : hf * half + (m + 1) * P,
                          n * 512 : (n + 1) * 512],
                in_=ysb[:],
            )

        # mm2 pass 0, one half: c-outer; w2c tiles preloaded and shared
        def mm2_pass0_half(w2cs, betas, hf):
            psy = [ps.tile([P, 512], f32, tag="ps", name=f"psy0_{hf}_{m}")
                   for m in range(mt)]
            for c in range(cc):
                hq = hq_tiles[hf][c]
                for m in range(mt):
                    nc.tensor.matmul(
                        psy[m][:], hq[:, m * P : (m + 1) * P],
                        w2cs[c][:], start=(c == 0), stop=(c == cc - 1),
                    )
            for m in range(mt):
                ysb_out(psy[m][:], betas, 0, hf, m, on_act=(m % 2 == 0))

        # ---------------- schedule ------------------------------------------
        # Back-split: half-0 finishes TAIL chunks early; tscale(0) PE bits
        # interleave into half-1's tail so its chain hides under matmuls.
        # mm2 pass 0 runs the halves as separate c-outer blocks: tscale(1)
        # and the quant streams hide under the blocks' matmuls.
        TAIL = 4
        for c in range(cc - TAIL):
            mm1_chunk(c, [0, 1])
        for c in range(cc - TAIL, cc):
            mm1_chunk(c, [0], keep_w1=True)
        t0_gen = tscale(0)
        w2cs0 = [w2_load(0, c) for c in range(cc)]
        for i, c in enumerate(range(cc - TAIL, cc)):
            mm1_chunk(c, [1])
            if i == 0:
                next(t0_gen)
            elif i == 1:
                beta0, shbc0 = next(t0_gen)
            elif i == 2:
                for q in range(3):
                    quant_chunk(0, q, shbc0)
            else:
                for q in range(3, 6):
                    quant_chunk(0, q, shbc0)
        betas = {0: beta0, 1: None}
        # pass 0, half 0
        psy0 = [ps.tile([P, 512], f32, tag="ps", name=f"psy0_0_{m}")
                for m in range(mt)]
        t1_gen = tscale(1)
        shbc1 = None
        for c in range(cc):
            for m in range(mt):
                nc.tensor.matmul(
                    psy0[m][:], hq_tiles[0][c][:, m * P : (m + 1) * P],
                    w2cs0[c][:], start=(c == 0), stop=(c == cc - 1),
                )
            if c == 0:
                next(t1_gen)
            elif c == 1:
                beta1, shbc1 = next(t1_gen)
                betas[1] = beta1
            elif c - 2 + 6 < cc:
                quant_chunk(0, c - 2 + 6, shbc0)
            elif c >= cc - 4:
                quant_chunk(1, c - (cc - 4), shbc1)
        for m in range(mt):
            ysb_out(psy0[m][:], betas, 0, 0, m, on_act=True)
        # pass 0, half 1, quant(1) interleaved at lag 4
        psy1 = [ps.tile([P, 512], f32, tag="ps", name=f"psy0_1_{m}")
                for m in range(mt)]
        for c in range(cc):
            for m in range(mt):
                nc.tensor.matmul(
                    psy1[m][:], hq_tiles[1][c][:, m * P : (m + 1) * P],
                    w2cs0[c][:], start=(c == 0), stop=(c == cc - 1),
                )
            if c + 4 < cc:
                quant_chunk(1, c + 4, shbc1)
        for m in range(mt):
            ysb_out(psy1[m][:], betas, 0, 1, m, on_act=True)
        # pass 1: m-paired groups (4 PSUM banks each, short output tail)
        w2cs = [w2_load(1, c) for c in range(cc)]
        for mp in range(2):
            psy = [[ps.tile([P, 512], f32, tag="ps",
                            name=f"psy1_{hf}_{2 * mp + mi}")
                    for mi in range(2)] for hf in range(2)]
            for c in range(cc):
                for hf in range(2):
                    for mi in range(2):
                        m = 2 * mp + mi
                        nc.tensor.matmul(
                            psy[hf][mi][:],
                            hq_tiles[hf][c][:, m * P : (m + 1) * P],
                            w2cs[c][:], start=(c == 0), stop=(c == cc - 1),
                        )
            for hf in range(2):
                for mi in range(2):
                    ysb_out(psy[hf][mi][:], betas, 1, hf, 2 * mp + mi,
                            on_act=(hf == 0))
    return nc


def _host_prep(x, w1, conv_w, conv_b, w2, t_own):
    """Quantize weights and build per-core halo-padded pre-scaled x slabs."""
    fp16 = np.float16
    cc, dc = H // P, D // P
    s1inv = np.maximum(np.mean(np.abs(w1)), np.float32(EPS)).astype(np.float32)
    w1q = np.clip(np.rint(w1 * (np.float32(1.0) / s1inv)), -1, 1).astype(
        np.float32
    )
    s2inv = np.maximum(np.mean(np.abs(w2)), np.float32(EPS)).astype(np.float32)
    w2q = np.clip(np.rint(w2 * (np.float32(1.0) / s2inv)), -1, 1).astype(
        np.float32
    )

    # w1s[c, p, k*128+m] = w1q[c*128+m, k*128+p] -> per-chunk contiguous lhsT
    w1s = np.ascontiguousarray(
        w1q.reshape(cc, P, dc, P).transpose(0, 3, 2, 1).reshape(cc, P, D)
    ).astype(fp16)
    w2t = np.ascontiguousarray(w2q.T).astype(fp16)          # [H, D]
    cw0 = conv_w[:, 0, 0].astype(np.float32)
    cw1 = conv_w[:, 0, 1].astype(np.float32)
    cw2 = conv_w[:, 0, 2].astype(np.float32)
    # folded conv: conv = cw1*(deq1 + r0*deq0 + r2*deq2); silu scale = cw1
    r0 = cw0 / cw1
    r2 = cw2 / cw1
    # overflow guard (DVE computes stt in f32 internally; this only bounds
    # the f32 products; error negligible since the corresponding cw0/cw2
    # contribution is then ~unchanged)
    lim = np.float32(2.0e4 * 3.0)
    r0 = np.clip(r0, -lim, lim)
    r2 = np.clip(r2, -lim, lim)
    cw = np.stack([r0, r2, cw1, conv_b.astype(np.float32)], axis=1)
    cwal = np.ascontiguousarray(
        cw.reshape(cc, P, 4).transpose(1, 0, 2).reshape(P, cc * 4)
    ).astype(np.float32)

    n_cores = x.shape[0] * x.shape[1] // t_own
    xf = x.reshape(-1, x.shape[-1]).astype(np.float32)
    am = np.abs(xf).max(axis=1, keepdims=True).astype(np.float32)
    amc = np.maximum(am, np.float32(EPS))
    sxv = (np.float32(1.0) / amc).astype(np.float32) * np.float32(127.0)
    xq = np.rint((xf * sxv).astype(np.float32)).astype(np.float32)
    alpha_row = (amc[:, 0] * np.float32(s1inv / 127.0)).astype(np.float32)
    xsc = (xq * alpha_row[:, None]).astype(fp16)   # pre-scaled activations
    slabs = []
    for c in range(n_cores):
        lo = c * t_own
        xe = np.zeros((t_own + 2, xf.shape[1]), fp16)
        xe[1 : 1 + t_own] = xsc[lo : lo + t_own]
        if lo % S != 0:
            xe[0] = xsc[lo - 1]
        if (lo + t_own) % S != 0 and lo + t_own < xf.shape[0]:
            xe[1 + t_own] = xsc[lo + t_own]
        xt = xe.T.reshape(dc, P, t_own + 2).transpose(1, 0, 2)
        slabs.append(np.ascontiguousarray(xt))
    beta_c = float(s2inv) / 127.0
    return w1s, w2t, cwal, slabs, beta_c


def _run(x, w1, conv_w, conv_b, w2, trace=False, **spmd_kwargs):
    import sys
    if "/opt/trn_rl_repo" not in sys.path:
        sys.path.append("/opt/trn_rl_repo")
    _install_tile_patch()
    from concourse.bass_utils import run_bass_kernel_spmd

    t_own = x.shape[0] * x.shape[1] // N_CORES
    w1s, w2t, cwal, slabs, beta_c = _host_prep(
        x, w1, conv_w, conv_b, w2, t_own
    )
    nc = build_nc(t_own, beta_c)
    mt = t_own // 2 // 128
    oh4 = np.zeros((mt, mt * 128), np.float32)
    for m in range(mt):
        oh4[m, m * 128 : (m + 1) * 128] = 127.0
    in_maps = [
        {"xst": slabs[c], "w1s": w1s, "w2t": w2t, "cwal": cwal, "oh4": oh4}
        for c in range(N_CORES)
    ]
    out = run_bass_kernel_spmd(
        nc, in_maps, list(range(N_CORES)), trace=trace, **spmd_kwargs
    )
    y = np.concatenate([out.results[c]["y"] for c in range(N_CORES)], axis=0)
    y = np.ascontiguousarray(y.reshape(x.shape[0], x.shape[1], -1))
    return y, out


def kernel(x, w1, conv_w, conv_b, w2):
    return _run(x, w1, conv_w, conv_b, w2)[0]
